# revision 9
# baseline (speedup 1.0000x reference)
"""Cross-attention kernel for TRN2, batch-parallel over 8 NeuronCores.

Problem shapes (hardcoded): B=8, C1=C2=256, H=W=32 (S=1024), NH=8, KD=VD=64.

Per-core program (core b computes batch element b, no collectives):
  X1 = input1[b] as [C1, S1] (natural layout), X2 likewise.
  K1T = Wk1 @ X1   -> [512, S1]   (head h rows h*64:(h+1)*64)   f32r matmul
  K2T = Wk2 @ X2   -> [512, S2]
  V2  = X2.T @ Wv2.T -> [S2, 512] natural layout, stored per-head with a
        ones column appended ([128, 8, 65] per s2-chunk, bf16)
  per head h:
    scoresT tiles [s2_blk=128, q=1024] = K2T[h][:,blk].T @ K1T[h]  (bf16)
    expT = exp(scoresT / 8)  (ScalarE, from PSUM, no max-subtraction: the
           scores are O(1) so plain exp matches softmax exactly)
    outT_aug [65, q] = sum_blk [v2|1].T @ expT_blk ; row 64 = softmax denom
    out_h = outT_aug[0:64] * (1/denom) broadcast  -> oall_h [64, q] f32
  finalT [C1, S1] = WoT.T @ concat_h(oall_h)  (f32r, K=64 per-head chunks)
  y = finalT reshaped [C1, H, W]  == output[b] layout exactly.
"""

import sys

for _p in ("/opt/trn_rl_repo", "/root/.axon_site/_ro/trn_rl_repo"):
    if _p not in sys.path:
        sys.path.append(_p)

import numpy as np

import concourse.bass as bass
import concourse.mybir as mybir
import concourse.tile as tile
from concourse import bacc, bass_utils

F32 = mybir.dt.float32
F32R = mybir.dt.float32r
BF16 = mybir.dt.bfloat16

B = 8
C1 = 256
S1 = 1024
C2 = 256
S2 = 1024
NH = 8
KD = 64
VD = 64
P = 128


def build_nc(dump=False):
    nc = bacc.Bacc(
        "TRN2",
        target_bir_lowering=False,
        debug=False,
        enable_asserts=False,
        num_devices=B,
    )

    x1 = nc.dram_tensor("x1", [C1, S1], F32R, kind="ExternalInput").ap()
    x2 = nc.dram_tensor("x2", [C2, S2], F32R, kind="ExternalInput").ap()
    wk1t = nc.dram_tensor("wk1t", [C1, NH * KD], F32R, kind="ExternalInput").ap()
    wk2t = nc.dram_tensor("wk2t", [C2, NH * KD], F32R, kind="ExternalInput").ap()
    wv2t = nc.dram_tensor("wv2t", [C2, NH * VD], F32R, kind="ExternalInput").ap()
    wot = nc.dram_tensor("wot", [NH * VD, C1], F32R, kind="ExternalInput").ap()
    y = nc.dram_tensor("y", [C1, S1], F32, kind="ExternalOutput").ap()
    dumps = {}
    if dump:
        for nm, shape in (
            ("d_k1t", [P, S1]), ("d_k2t", [P, S1]), ("d_v2a", [P, NH * (VD + 1)]),
            ("d_qk0", [P, S1]), ("d_qk1", [P, S1]), ("d_expt", [P, S1]),
            ("d_av", [VD + 1, S1]), ("d_recip", [1, S1]), ("d_rep", [64, S1]),
            ("d_oall", [64, S1]),
        ):
            dumps[nm] = nc.dram_tensor(nm, shape, F32, kind="ExternalOutput").ap()

    with tile.TileContext(nc) as tc:
        with (
            tc.tile_pool(name="const", bufs=1) as cpool,
            tc.tile_pool(name="expt", bufs=10) as epool,
            tc.tile_pool(name="norm", bufs=2) as npool,
            tc.tile_pool(name="yout", bufs=2) as ypool,
            tc.tile_pool(name="pmm", bufs=2, space="PSUM") as pmm,
            tc.tile_pool(name="pav", bufs=2, space="PSUM") as pav,
            tc.tile_pool(name="dscr", bufs=2, space="DRAM") as dpool,
        ):
            # ---- load inputs ----
            x1_sb = [cpool.tile([P, S1], F32R, name=f"x1_{c}") for c in range(2)]
            x2_sb = [cpool.tile([P, S2], F32R, name=f"x2_{c}") for c in range(2)]
            wk1t_sb = [cpool.tile([P, 512], F32R, name=f"wk1t_{c}") for c in range(2)]
            wk2t_sb = [cpool.tile([P, 512], F32R, name=f"wk2t_{c}") for c in range(2)]
            wv2t_sb = [cpool.tile([P, 512], F32R, name=f"wv2t_{c}") for c in range(2)]
            wot_sb = [cpool.tile([64, C1], F32R, name=f"wot_{h}") for h in range(NH)]
            for c in range(2):
                nc.sync.dma_start(x1_sb[c][:], x1[c * P : (c + 1) * P, :])
                nc.sync.dma_start(x2_sb[c][:], x2[c * P : (c + 1) * P, :])
                nc.sync.dma_start(wk1t_sb[c][:], wk1t[c * P : (c + 1) * P, :])
                nc.sync.dma_start(wk2t_sb[c][:], wk2t[c * P : (c + 1) * P, :])
                nc.sync.dma_start(wv2t_sb[c][:], wv2t[c * P : (c + 1) * P, :])
            for h in range(NH):
                nc.sync.dma_start(wot_sb[h][:], wot[h * 64 : (h + 1) * 64, :])

            k1t_sb = [cpool.tile([P, S1], BF16, name=f"k1t_{m}") for m in range(4)]
            k2t_sb = [cpool.tile([P, S2], BF16, name=f"k2t_{m}") for m in range(4)]
            # v2 with per-head ones column: [128, head, 65]
            v2a_sb = [
                cpool.tile([P, NH, VD + 1], BF16, name=f"v2a_{s}") for s in range(8)
            ]
            oall_sb = [cpool.tile([64, S1], F32R, name=f"oall_{h}") for h in range(NH)]

            # ---- K1T / K2T projections: out[m] = (WkT chunk).T @ X chunks ----
            for wt_sb, xs_sb, kt_sb in (
                (wk1t_sb, x1_sb, k1t_sb),
                (wk2t_sb, x2_sb, k2t_sb),
            ):
                for m in range(4):
                    ps = pmm.tile([P, 1024], F32, tag="qk", name=f"ps_proj_{m}")
                    for nh_ in range(2):
                        for k in range(2):
                            nc.tensor.matmul(
                                ps[:, nh_ * 512 : (nh_ + 1) * 512],
                                wt_sb[k][:, m * P : (m + 1) * P],
                                xs_sb[k][:, nh_ * 512 : (nh_ + 1) * 512],
                                start=(k == 0),
                                stop=(k == 1),
                            )
                    nc.vector.tensor_copy(out=kt_sb[m][:], in_=ps[:])
                    if dump and m == 0:
                        dt_ = ypool.tile([P, S1], F32, tag="dmp", name=f"dk_{m}")
                        nc.vector.tensor_copy(out=dt_[:], in_=kt_sb[0][:])
                        nc.sync.dma_start(
                            dumps["d_k1t" if kt_sb is k1t_sb else "d_k2t"], dt_[:]
                        )

            # ---- V2 natural layout (+ ones col) ----
            for sp in range(4):  # pairs of s2 chunks
                ps = pmm.tile([P, 1024], F32, tag="qk", name=f"ps_v2_{sp}")
                for half in range(2):
                    s = 2 * sp + half
                    for k in range(2):
                        nc.tensor.matmul(
                            ps[:, half * 512 : (half + 1) * 512],
                            x2_sb[k][:, s * P : (s + 1) * P],
                            wv2t_sb[k][:],
                            start=(k == 0),
                            stop=(k == 1),
                        )
                for half in range(2):
                    s = 2 * sp + half
                    nc.vector.memset(v2a_sb[s][:, :, VD : VD + 1], 1.0)
                    nc.vector.tensor_copy(
                        out=v2a_sb[s][:, :, 0:VD],
                        in_=ps[:, half * 512 : (half + 1) * 512].rearrange(
                            "p (h c) -> p h c", c=VD
                        ),
                    )
                    if dump and s == 0:
                        dt_ = ypool.tile([P, NH * (VD + 1)], F32, tag="dmp2", name="dv2a")
                        nc.vector.tensor_copy(
                            out=dt_[:].rearrange("p (h c) -> p h c", c=VD + 1),
                            in_=v2a_sb[0][:],
                        )
                        nc.sync.dma_start(dumps["d_v2a"], dt_[:])

            # ---- attention per head ----
            for h in range(NH):
                m = h // 2
                ro = (h % 2) * 64  # row offset of this head inside chunk m

                expts = []
                for s2 in range(8):
                    qk_ps = pmm.tile([P, S1], F32, tag="qk", name=f"qk_{h}_{s2}")
                    lhsT = k2t_sb[m][ro : ro + 64, s2 * P : (s2 + 1) * P]
                    for nh_ in range(2):
                        nc.tensor.matmul(
                            qk_ps[:, nh_ * 512 : (nh_ + 1) * 512],
                            lhsT,
                            k1t_sb[m][ro : ro + 64, nh_ * 512 : (nh_ + 1) * 512],
                            start=True,
                            stop=True,
                        )
                    if dump and h in (0, 1) and s2 == 0:
                        dt_ = ypool.tile([P, S1], F32, tag="dmp", name=f"dqk_{h}")
                        nc.vector.tensor_copy(out=dt_[:], in_=qk_ps[:])
                        nc.sync.dma_start(dumps[f"d_qk{h}"], dt_[:])
                    et = epool.tile([P, S1], BF16, tag="expt", name=f"expt_{h}_{s2}")
                    nc.scalar.activation(
                        et[:], qk_ps[:], mybir.ActivationFunctionType.Exp, scale=0.125
                    )
                    expts.append(et)
                    if dump and h == 0 and s2 == 0:
                        dt_ = ypool.tile([P, S1], F32, tag="dmp", name="dexpt")
                        nc.vector.tensor_copy(out=dt_[:], in_=et[:])
                        nc.sync.dma_start(dumps["d_expt"], dt_[:])

                av_ps = pav.tile([VD + 1, S1], F32, tag="av", name=f"av_{h}")
                for nh_ in range(2):
                    for s2 in range(8):
                        nc.tensor.matmul(
                            av_ps[:, nh_ * 512 : (nh_ + 1) * 512],
                            v2a_sb[s2][:, h, :],
                            expts[s2][:, nh_ * 512 : (nh_ + 1) * 512],
                            start=(s2 == 0),
                            stop=(s2 == 7),
                        )

                if dump and h == 0:
                    dt_ = ypool.tile([VD + 1, S1], F32, tag="dmp", name="dav")
                    nc.vector.tensor_copy(out=dt_[:], in_=av_ps[:])
                    nc.sync.dma_start(dumps["d_av"], dt_[:])
                # normalize: oall_h = av[0:64] * (1/denom), denom = av[64]
                recip = npool.tile([VD + 1, S1], F32, tag="recip", name=f"recip_{h}")
                nc.vector.reciprocal(recip[VD : VD + 1, :], av_ps[VD : VD + 1, :])
                rdram = dpool.tile([S1], F32, tag="rd", name=f"rd_{h}")
                nc.sync.dma_start(rdram[:], recip[VD : VD + 1, :])
                rep = npool.tile([64, S1], F32, tag="rep", name=f"rep_{h}")
                nc.sync.dma_start(rep[:], rdram[None, :].to_broadcast((64, S1)))
                nc.vector.tensor_mul(
                    out=oall_sb[h][:], in0=av_ps[0:VD, :], in1=rep[:]
                )
                if dump and h == 0:
                    nc.sync.dma_start(dumps["d_recip"], recip[VD : VD + 1, :])
                    nc.sync.dma_start(dumps["d_rep"], rep[:])
                    dt_ = ypool.tile([64, S1], F32, tag="dmp", name="doall")
                    nc.vector.tensor_copy(out=dt_[:], in_=oall_sb[0][:])
                    nc.sync.dma_start(dumps["d_oall"], dt_[:])

            # ---- final projection: y[mt] = sum_h WoT_h.T @ oall_h ----
            for mt in range(2):
                fin = pmm.tile([P, S1], F32, tag="qk", name=f"fin_{mt}")
                for nh_ in range(2):
                    for h in range(NH):
                        nc.tensor.matmul(
                            fin[:, nh_ * 512 : (nh_ + 1) * 512],
                            wot_sb[h][:, mt * P : (mt + 1) * P],
                            oall_sb[h][:, nh_ * 512 : (nh_ + 1) * 512],
                            start=(h == 0),
                            stop=(h == NH - 1),
                        )
                ysb = ypool.tile([P, S1], F32, tag="y", name=f"y_{mt}")
                nc.vector.tensor_copy(out=ysb[:], in_=fin[:])
                nc.sync.dma_start(y[mt * P : (mt + 1) * P, :], ysb[:])

    nc.compile()
    return nc


_nc_cache = None


def _get_nc():
    global _nc_cache
    if _nc_cache is None:
        _nc_cache = build_nc()
    return _nc_cache


def _make_in_maps(input1, input2, Wk1, Wk2, Wv2, Wo):
    input1 = np.ascontiguousarray(np.asarray(input1, dtype=np.float32))
    input2 = np.ascontiguousarray(np.asarray(input2, dtype=np.float32))
    wk1t = np.ascontiguousarray(np.asarray(Wk1, dtype=np.float32).T)
    wk2t = np.ascontiguousarray(np.asarray(Wk2, dtype=np.float32).T)
    wv2t = np.ascontiguousarray(np.asarray(Wv2, dtype=np.float32).T)
    wot = np.ascontiguousarray(np.asarray(Wo, dtype=np.float32).T)
    return [
        {
            "x1": np.ascontiguousarray(input1[b].reshape(C1, S1)),
            "x2": np.ascontiguousarray(input2[b].reshape(C2, S2)),
            "wk1t": wk1t,
            "wk2t": wk2t,
            "wv2t": wv2t,
            "wot": wot,
        }
        for b in range(B)
    ]


def _assemble(results):
    out = np.stack([results[b]["y"] for b in range(B)], axis=0)
    return np.ascontiguousarray(out.reshape(B, C1, 32, 32).astype(np.float32))


def kernel(input1, input2, Wk1, Wk2, Wv2, Wo):
    nc = _get_nc()
    in_maps = _make_in_maps(input1, input2, Wk1, Wk2, Wv2, Wo)
    res = bass_utils.run_bass_kernel_spmd(nc, in_maps, core_ids=list(range(B)))
    return _assemble(res.results)


def kernel_traced(input1, input2, Wk1, Wk2, Wv2, Wo):
    """Like kernel() but with NTFF profiling; returns (out, BassKernelResults)."""
    nc = _get_nc()
    in_maps = _make_in_maps(input1, input2, Wk1, Wk2, Wv2, Wo)
    res = bass_utils.run_bass_kernel_spmd(
        nc, in_maps, core_ids=list(range(B)), trace=True
    )
    return _assemble(res.results), res
